# revision 6
# baseline (speedup 1.0000x reference)
"""NeuralGCDE Trainium2 kernel.

Sharding: data-parallel over batch (B=16 -> 2 per core x 8 cores). Each core
integrates the full RK4 ODE (11 steps x 4 vector-field evals) for its 614
tokens (2 batches x 307 nodes) in a feature-major layout (features on SBUF
partitions, tokens on the free dim), so every weight application is a
weight-stationary matmul and biases ride along as ones-row / ACT-bias tricks.

Algebraic restructurings (validated to 2.5e-7 vs the reference in fp32):
  * spline derivatives dX at the 33 distinct (step, offset) eval points are
    precomputed on host (they don't depend on the evolving state).
  * dz = einsum('bnij,bnj->bni', g_v, dh) replaces the fg tensor.
  * the per-node adaptive graph conv is expanded over the embedding dim:
    x2 = sum_d (x_g * gE_d) @ Wpool_d (+ rank-8 bias via gE @ g_bpool).
  * tanh(g_out) is produced in 32 feature chunks of 128 partitions
    ((i-pair, j) layout); dz's per-token contraction over j becomes an
    elementwise multiply by a duplicated dh followed by 0/1-selector
    matmuls that reduce over the partition axis, accumulating in PSUM.

All matmuls/elementwise run in fp16 (1.4e-4 rel err vs reference, validated
in numpy emulation); carried ODE states and PSUM accumulation stay fp32.
"""

import sys

for _p in ("/opt/trn_rl_repo", "/root/.axon_site/_ro/trn_rl_repo"):
    if _p not in sys.path:
        sys.path.append(_p)

import numpy as np

B, N, T, C, H, HH, ED, HOR, OC = 16, 307, 12, 2, 64, 64, 8, 12, 1
NC_COUNT = 8
BL = B // NC_COUNT          # local batches per core
TK = BL * N                 # tokens per core (614)
HTK = N                     # token half = one local batch (307)
NSTEP = T - 1               # 11
NSLICE = 3 * NSTEP          # 33 distinct dX slices
NCH = (H * H) // 128        # 32 g_out chunks of 128 features

_CACHE = {}


def _np16(x):
    return np.ascontiguousarray(x, dtype=np.float16)


def _np32(x):
    return np.ascontiguousarray(x, dtype=np.float32)


def _build_consts(inp):
    """Host preprocessing of the replicated (core-independent) constants."""
    gE = _np32(inp["g_E"])                                    # (N, ED)

    logits = np.maximum(gE @ gE.T, 0.0)
    e = np.exp(logits - logits.max(axis=1, keepdims=True))
    A = e / e.sum(axis=1, keepdims=True)                      # (N, N)
    at = np.zeros((128, 3 * N), np.float16)
    for c in range(3):
        mc = min(128, N - c * 128)
        at[:mc, c * N:c * N + N] = A.T[c * 128:c * 128 + mc, :]

    wf1 = np.concatenate([_np32(inp["f_W_in"]), _np32(inp["f_b_in"])[None, :]], 0)
    wf2 = np.concatenate([_np32(inp["f_W_mid"]), _np32(inp["f_b_mid"])[None, :]], 0)
    # f_W_out columns permuted so fv partition p = c*64 + h
    perm = np.empty(H * C, np.int64)
    for cc in range(C):
        for hh in range(H):
            perm[cc * H + hh] = hh * C + cc
    wf3 = _np32(inp["f_W_out"])[:, perm]                      # (64, 128)
    bf3 = _np32(inp["f_b_out"])[perm][:, None]                # (128, 1)
    wg1 = np.concatenate([_np32(inp["g_W_in"]), _np32(inp["g_b_in"])[None, :]], 0)

    # Wpool chunks arranged (d, (k,i), o)
    wpool = np.zeros((128, ED * HH), np.float16)
    gwp = _np32(inp["g_Wpool"])                               # (ED, 2, HH, HH)
    for d in range(ED):
        wpool[0:HH, d * HH:(d + 1) * HH] = gwp[d, 0]
        wpool[HH:2 * HH, d * HH:(d + 1) * HH] = gwp[d, 1]
    gbp = _np16(inp["g_bpool"])                               # (ED, HH)

    wgo = _np16(inp["g_W_out"])                               # (64, 4096)
    gbo = _np32(inp["g_b_out"]).reshape(NCH, 128).T.copy()    # (128, NCH)

    ipair = np.zeros((128, H), np.float16)
    for p in range(128):
        ipair[p, p % H] = 1.0

    return dict(
        at=at, wf1=_np16(wf1), wf2=_np16(wf2), wf3=_np16(wf3), bf3=_np32(bf3),
        wg1=_np16(wg1), wpool=wpool, gbp=gbp, wgo=wgo, gbo=gbo,
        ipair=ipair,
    ), A, gE


def _build_core_inputs(inp, gE, consts):
    """Per-core inputs: dX slices (broadcast layout), gE-per-token, h0/z0."""
    cb, cc, cd = _np32(inp["coeff_b"]), _np32(inp["coeff_c"]), _np32(inp["coeff_d"])
    ca = _np32(inp["coeff_a"])

    dX = np.zeros((NSTEP, 3, B, N, C), np.float32)
    for i in range(NSTEP):
        dX[i, 0] = cb[:, :, i]
        dX[i, 1] = cb[:, :, i] + 0.5 * cc[:, :, i] + 0.25 * cd[:, :, i]
        if i < NSTEP - 1:
            dX[i, 2] = cb[:, :, i + 1]
        else:
            dX[i, 2] = cb[:, :, i] + cc[:, :, i] + cd[:, :, i]

    x0 = ca[:, :, 0, :]
    h0 = x0 @ _np32(inp["h_W"]) + _np32(inp["h_b"])           # (B, N, H)
    z0 = x0 @ _np32(inp["z_W"]) + _np32(inp["z_b"])

    getok = np.zeros((ED, TK), np.float16)
    for lb in range(BL):
        getok[:, lb * N:(lb + 1) * N] = gE.T
    maps = []
    for ci in range(NC_COUNT):
        b0 = ci * BL
        dxs = np.zeros((2, NSLICE * TK), np.float16)
        for s in range(NSTEP):
            for e0 in range(3):
                flat = dX[s, e0, b0:b0 + BL].reshape(TK, C)
                col = (3 * s + e0) * TK
                dxs[0, col:col + TK] = flat[:, 0]
                dxs[1, col:col + TK] = flat[:, 1]
        h0t = h0[b0:b0 + BL].reshape(TK, H).T.copy()          # (64, TK)
        z0t = z0[b0:b0 + BL].reshape(TK, H).T.copy()
        maps.append(dict(
            dxs=dxs, h0=_np32(h0t), z0=_np32(z0t),
            getok=getok, **consts,
        ))
    return maps


def _build_kernel(n_steps=NSTEP):
    import concourse.bass as bass  # noqa: F401
    import concourse.mybir as mybir
    from concourse import bacc, tile

    F16 = mybir.dt.float16
    F32 = mybir.dt.float32
    AF = mybir.ActivationFunctionType
    OP = mybir.AluOpType

    nc = bacc.Bacc("TRN2", target_bir_lowering=False, debug=False,
                   enable_asserts=True, num_devices=NC_COUNT)

    dr = {}
    for name, shape, dt in [
        ("wf1", (65, 64), F16), ("wf2", (65, 64), F16),
        ("wf3", (64, 128), F16), ("bf3", (128, 1), F32),
        ("wg1", (65, 64), F16), ("at", (128, 3 * N), F16),
        ("wpool", (128, ED * HH), F16), ("gbp", (ED, HH), F16),
        ("wgo", (64, NCH * 128), F16), ("gbo", (128, NCH), F32),
        ("ipair", (128, H), F16),
        ("getok", (ED, TK), F16),
        ("dxs", (2, NSLICE * TK), F16),
        ("h0", (64, TK), F32), ("z0", (64, TK), F32),
    ]:
        dr[name] = nc.dram_tensor(name, shape, dt, kind="ExternalInput")
    zout_d = nc.dram_tensor("zout", (64, TK), F32, kind="ExternalOutput")

    with tile.TileContext(nc) as tc:
        with tc.tile_pool(name="consts", bufs=1) as pc, \
             tc.tile_pool(name="work", bufs=1) as pw, \
             tc.tile_pool(name="psum", bufs=1, space="PSUM") as pp:

            ct = {}
            for name in ("wf1", "wf2", "wf3", "bf3", "wg1", "at", "wpool",
                         "gbp", "wgo", "gbo", "ipair", "getok"):
                d = dr[name]
                t = pc.tile(list(d.shape), d.dtype, tag=name)
                nc.sync.dma_start(t[:], d[:])
                ct[name] = t
            # broadcast-fill dxb (128, NSLICE*TK) from compact dxs (2, .)
            dxb_t = pc.tile([128, NSLICE * TK], F16, tag="dxb")
            for bb in range(2):
                nc.sync.dma_start(
                    dxb_t[64 * bb:64 * (bb + 1), :],
                    dr["dxs"][bb:bb + 1, :].broadcast_to((64, NSLICE * TK)))
            ct["dxb"] = dxb_t
            # broadcast-fill gebb (128, ED*TK) from getok (ED, TK)
            gebb_t = pc.tile([128, ED * TK], F16, tag="gebb")
            nc.sync.dma_start(
                gebb_t[:].rearrange("p (d t) -> p d t", d=ED),
                dr["getok"][:].unsqueeze(0).broadcast_to((128, ED, TK)))
            ct["gebb"] = gebb_t
            # build the 0/1 dz selector in place
            sel_t = pc.tile([128, NCH * H], F16, tag="sel")
            nc.gpsimd.memset(sel_t[:], 0.0)
            for c in range(NCH):
                nc.gpsimd.memset(sel_t[0:64, c * H + 2 * c:c * H + 2 * c + 1], 1.0)
                nc.gpsimd.memset(
                    sel_t[64:128, c * H + 2 * c + 1:c * H + 2 * c + 2], 1.0)
            ct["sel"] = sel_t

            h32 = pw.tile([64, TK], F32, tag="h32")
            z32 = pw.tile([64, TK], F32, tag="z32")
            hrun = pw.tile([64, TK], F32, tag="hrun")
            zrun = pw.tile([64, TK], F32, tag="zrun")
            hs16 = pw.tile([65, TK], F16, tag="hs16")
            zs16 = pw.tile([65, TK], F16, tag="zs16")
            dh32 = pw.tile([64, TK], F32, tag="dh32")
            htmp = pw.tile([64, TK], F32, tag="htmp")
            dht2 = pw.tile([128, TK], F16, tag="dht2")
            x1f = pw.tile([65, TK], F16, tag="x1f")
            x2f = pw.tile([64, TK], F16, tag="x2f")
            fv = pw.tile([128, TK], F16, tag="fv")
            ftmp = pw.tile([128, TK], F16, tag="ftmp")
            xg = pw.tile([128, 2 * 384], F16, tag="xg")  # 384-wide half-slots (padded for xbar transpose)
            xbt = pw.tile([128, 2 * 3 * 64], F16, tag="xbt")
            zexp = pw.tile([128, ED * TK], F16, tag="zexp")
            xo = pw.tile([64, TK], F16, tag="xo")
            gv = pw.tile([128, 2 * NCH * HTK], F16, tag="gv")

            ps = pp.tile([128, 4096], F32, tag="ps")

            # PSUM bank map (fp32-element offsets; bank = 512):
            #   banks 0-3: g_out stream slots (pairs (0,1) / (2,3))
            #   bank 4: f-path chain   bank 5: g-head chain
            #   banks 6,7: dz halves
            GSLOT = (0, 512, 1024, 1536)
            FCH = 2048
            GCH = 2560
            DZ0 = 3072

            def mm(out_ap, lhs_ap, rhs_ap, start=True, stop=True):
                nc.tensor.matmul(out_ap, lhs_ap, rhs_ap, start=start,
                                 stop=stop, skip_group_check=True)

            nc.gpsimd.memset(hs16[64:65, :], 1.0)
            nc.gpsimd.memset(zs16[64:65, :], 1.0)
            nc.gpsimd.memset(x1f[64:65, :], 1.0)
            nc.gpsimd.memset(xg[:], 0.0)
            nc.sync.dma_start(h32[:], dr["h0"][:])
            nc.sync.dma_start(z32[:], dr["z0"][:])
            nc.vector.tensor_copy(hs16[0:64, :], h32[:])
            nc.vector.tensor_copy(zs16[0:64, :], z32[:])

            dzap = ps[0:64, DZ0:DZ0 + 1024].rearrange(
                "p (a t) -> p a t", a=2, t=512)[:, :, 0:HTK]
            z32v = z32[:].rearrange("p (a t) -> p a t", a=2)
            zrunv = zrun[:].rearrange("p (a t) -> p a t", a=2)
            zs16v = zs16[0:64, :].rearrange("p (a t) -> p a t", a=2)
            dht2v = dht2[:].rearrange("p (a t) -> p a t", a=2)

            for s in range(n_steps):
                for stage in range(4):
                    e0 = (0, 1, 1, 2)[stage]
                    dxcol = (3 * s + e0) * TK

                    # ---------------- f path ----------------
                    for hh in range(2):
                        tk = slice(hh * HTK, (hh + 1) * HTK)
                        p_f = ps[0:64, FCH:FCH + HTK]
                        mm(p_f, ct["wf1"][:], hs16[:, tk])
                        nc.vector.tensor_scalar_max(x1f[0:64, tk], p_f, 0.0)
                        mm(p_f, ct["wf2"][:], x1f[:, tk])
                        nc.vector.tensor_scalar_max(x2f[:, tk], p_f, 0.0)
                        p_f3 = ps[0:128, FCH:FCH + HTK]
                        mm(p_f3, ct["wf3"][:], x2f[:, tk])
                        nc.scalar.activation(fv[:, tk], p_f3, AF.Tanh,
                                             bias=ct["bf3"][:])
                        nc.vector.tensor_mul(
                            ftmp[:, tk], fv[:, tk],
                            ct["dxb"][:, dxcol + hh * HTK:dxcol + (hh + 1) * HTK])
                        p_dh = ps[0:64, FCH:FCH + HTK]
                        mm(p_dh, ct["ipair"][:], ftmp[:, tk])
                        nc.vector.tensor_copy(dht2[0:64, tk], p_dh)
                        nc.vector.tensor_copy(dh32[:, tk], p_dh)
                        nc.vector.tensor_copy(dht2[64:128, tk], dht2[0:64, tk])

                    # RK4 h-side (gpsimd; no fused STT on Pool engine)
                    if stage == 0:
                        nc.gpsimd.tensor_scalar_mul(htmp[:], dh32[:], 1.0 / 6.0)
                        nc.gpsimd.tensor_add(hrun[:], htmp[:], h32[:])
                        nc.gpsimd.tensor_scalar_mul(htmp[:], dh32[:], 0.5)
                        nc.gpsimd.tensor_add(hs16[0:64, :], htmp[:], h32[:])
                    elif stage in (1, 2):
                        nc.gpsimd.tensor_scalar_mul(htmp[:], dh32[:], 1.0 / 3.0)
                        nc.gpsimd.tensor_add(hrun[:], htmp[:], hrun[:])
                        nc.gpsimd.tensor_scalar_mul(
                            htmp[:], dh32[:], 0.5 if stage == 1 else 1.0)
                        nc.gpsimd.tensor_add(hs16[0:64, :], htmp[:], h32[:])
                    else:
                        nc.gpsimd.tensor_scalar_mul(htmp[:], dh32[:], 1.0 / 6.0)
                        nc.gpsimd.tensor_add(hs16[0:64, :], htmp[:], hrun[:])
                        nc.gpsimd.tensor_add(h32[:], htmp[:], hrun[:])

                    # ---------------- g path head ----------------
                    for hh in range(2):
                        tk = slice(hh * HTK, (hh + 1) * HTK)
                        xgs = slice(hh * 384, hh * 384 + HTK)
                        p_g1 = ps[0:64, GCH:GCH + HTK]
                        mm(p_g1, ct["wg1"][:], zs16[:, tk])
                        nc.vector.tensor_scalar_max(xg[0:64, xgs], p_g1, 0.0)
                        for c in range(3):
                            nc.sync.dma_start_transpose(
                                xbt[:, (hh * 3 + c) * 64:(hh * 3 + c + 1) * 64],
                                xg[0:64,
                                   hh * 384 + c * 128:hh * 384 + (c + 1) * 128])
                        p_am = ps[0:64, GCH:GCH + HTK]
                        for c in range(3):
                            mc = min(128, N - c * 128)
                            mm(p_am,
                               xbt[0:mc, (hh * 3 + c) * 64:(hh * 3 + c + 1) * 64],
                               ct["at"][0:mc, c * N:(c + 1) * N],
                               start=(c == 0), stop=(c == 2))
                        nc.vector.tensor_scalar_max(xg[64:128, xgs], p_am, 0.0)
                        for d in range(ED):
                            eng = nc.vector if d % 2 == 0 else nc.gpsimd
                            eng.tensor_mul(
                                zexp[:, d * TK + hh * HTK:d * TK + (hh + 1) * HTK],
                                xg[:, xgs],
                                ct["gebb"][:, d * TK + hh * HTK:
                                           d * TK + (hh + 1) * HTK])
                        p_agc = ps[0:64, GCH:GCH + HTK]
                        for d in range(ED):
                            mm(p_agc, ct["wpool"][:, d * HH:(d + 1) * HH],
                               zexp[:, d * TK + hh * HTK:d * TK + (hh + 1) * HTK],
                               start=(d == 0), stop=False)
                        mm(p_agc, ct["gbp"][:], ct["getok"][0:ED, tk],
                           start=False, stop=True)
                        nc.vector.tensor_scalar_max(xo[:, tk], p_agc, 0.0)

                    # ------- g_out stream: mm -> tanh -> *dh -> dz -------
                    for c in range(NCH):
                        s_a = GSLOT[(2 * c) % 4]
                        mm(ps[0:128, s_a:s_a + HTK],
                           ct["wgo"][:, c * 128:(c + 1) * 128], xo[:, 0:HTK])
                        mm(ps[0:128, s_a + 512:s_a + 512 + HTK],
                           ct["wgo"][:, c * 128:(c + 1) * 128], xo[:, HTK:TK])
                        gvsl = gv[:, (2 * c) * HTK:(2 * c + 2) * HTK].rearrange(
                            "p (a t) -> p a t", a=2)
                        act_src = ps[0:128, s_a:s_a + 1024].rearrange(
                            "p (a t) -> p a t", a=2, t=512)[:, :, 0:HTK]
                        nc.scalar.activation(gvsl, act_src, AF.Tanh,
                                             bias=ct["gbo"][:, c:c + 1])
                        nc.vector.tensor_mul(gvsl, gvsl, dht2v)
                        mm(ps[0:64, DZ0:DZ0 + HTK],
                           ct["sel"][:, c * H:(c + 1) * H],
                           gv[:, (2 * c) * HTK:(2 * c + 1) * HTK],
                           start=(c == 0), stop=(c == NCH - 1))
                        mm(ps[0:64, DZ0 + 512:DZ0 + 512 + HTK],
                           ct["sel"][:, c * H:(c + 1) * H],
                           gv[:, (2 * c + 1) * HTK:(2 * c + 2) * HTK],
                           start=(c == 0), stop=(c == NCH - 1))

                    # RK4 z-side (vector: reads dz straight from PSUM)
                    if stage == 0:
                        nc.vector.scalar_tensor_tensor(
                            zrunv, dzap, 1.0 / 6.0, z32v,
                            op0=OP.mult, op1=OP.add)
                        nc.vector.scalar_tensor_tensor(
                            zs16v, dzap, 0.5, z32v, op0=OP.mult, op1=OP.add)
                    elif stage in (1, 2):
                        nc.vector.scalar_tensor_tensor(
                            zrunv, dzap, 1.0 / 3.0, zrunv,
                            op0=OP.mult, op1=OP.add)
                        nc.vector.scalar_tensor_tensor(
                            zs16v, dzap, 0.5 if stage == 1 else 1.0, z32v,
                            op0=OP.mult, op1=OP.add)
                    else:
                        nc.vector.scalar_tensor_tensor(
                            zs16v, dzap, 1.0 / 6.0, zrunv,
                            op0=OP.mult, op1=OP.add)
                        nc.vector.scalar_tensor_tensor(
                            z32v, dzap, 1.0 / 6.0, zrunv,
                            op0=OP.mult, op1=OP.add)

            nc.sync.dma_start(zout_d[:], z32[:])

    nc.compile()
    return nc


def kernel(**inputs):
    if "nc" not in _CACHE:
        _CACHE["nc"] = _build_kernel()
    nc = _CACHE["nc"]

    consts, A, gE = _build_consts(inputs)
    in_maps = _build_core_inputs(inputs, gE, consts)

    from concourse.bass_utils import run_bass_kernel_spmd
    res = run_bass_kernel_spmd(nc, in_maps, core_ids=list(range(NC_COUNT)))

    z = np.zeros((B, N, H), np.float32)
    for ci in range(NC_COUNT):
        zt = np.asarray(res.results[ci]["zout"], dtype=np.float32)
        z[ci * BL:(ci + 1) * BL] = zt.T.reshape(BL, N, H)

    out = np.einsum("bnh,oh->bon", z, _np32(inputs["conv_W"])) \
        + _np32(inputs["conv_b"])[None, :, None]
    out = out.reshape(B, HOR, OC, N).transpose(0, 1, 3, 2)
    return np.ascontiguousarray(out, dtype=np.float32)
